# revision 1
# baseline (speedup 1.0000x reference)
"""AggregatedAttention (TransNeXt-style 3x3 local window + pooled-global attention)
Trainium2 Bass/Tile kernel, batch-parallel over 8 NeuronCores.

Layout strategy: feature-on-partition ("transposed") activations qT/kT/vT
[DIM, N] so the 3x3 spatial window becomes free-axis AP offsets. Local QK
products and AV products run on DVE in bf16; per-head d-reductions, l-sums,
head-broadcasts and all dense GEMMs run on the tensor engine (f32r for fp32
operands at bf16 row rate). The joint softmax denominator (9 local + 49
pooled) is inverted once and folded into the AV weights, so attention output
accumulates fully-divided in PSUM.
"""

import numpy as np
import ml_dtypes

import concourse.bass as bass
import concourse.tile as tile
import concourse.mybir as mybir
from concourse import bacc
from concourse.bass_utils import run_bass_kernel_spmd

F32 = mybir.dt.float32
F32R = mybir.dt.float32r
BF16 = mybir.dt.bfloat16
AF = mybir.ActivationFunctionType

B, H, W, DIM, NH, WS, SR = 8, 56, 56, 256, 8, 3, 8
HD = DIM // NH          # 32
N = H * W               # 3136
L = WS * WS             # 9
PH = PW = H // SR       # 7
PL = PH * PW            # 49
SCALE = DIM ** -0.5     # 1/16
CH = 448                # tokens per chunk = 8 image rows
NCH = N // CH           # 7
PAD = 64                # k/v halo columns each side
NKV = PAD + N + PAD
OFFS = [(di * W + dj) for di in (-1, 0, 1) for dj in (-1, 0, 1)]
LN_EPS_S = 1e-5 * (SR * SR) ** 2   # LN on s = 64*xp: var_s = 4096*var_xp

# ---------------- fp32 const pack columns ----------------
C_BQ = 0        # 2 cols: (bq + qe)*SCALE
C_BKV = 2       # 4 cols
C_BSR = 6       # 2
C_BPROJ = 8     # 2
C_LNG = 10      # 2
C_LNB = 12      # 2
C_PBL = 14      # 1 col, 72 partitions: pos_bias_local arranged
C_PB2 = 15      # 4 cols, 98 partitions: pos_bias_pool per head-pair
C_E72 = 19      # 72 cols, rows 0-7: R -> [72] expander
C_E98 = 91      # 4*98 cols, rows 0-7: R -> [98] expander per head-pair
C_IDF = 483     # 128 cols: fp32 identity (PE transpose)
NF32 = C_IDF + 128
# ---------------- bf16 const pack columns ----------------
K_BO72 = 0      # 18*72 cols: per-(l,half) d-reduction lhsT, ones-blocks at col offset
K_BL = 18 * 72  # 8 cols, 72 rows: l-sum for local denominator
K_LT4 = K_BL + 8       # 2*72 cols: learnable_tokens/SCALE lhsT per half (M=72)
K_I128 = K_LT4 + 144   # 128 cols: bf16 identity
K_ON8 = K_I128 + 128   # 4*8 cols, 98 rows: pool denominator lhsT per head-pair
K_MASK = K_ON8 + 32    # 3*448 cols, 72 rows: validity masks (top/interior/bottom)
NBF = K_MASK + 3 * CH


def _host_consts(inputs):
    """Build the two packed constant arrays + dense weights (fp32)."""
    f = np.zeros((128, NF32), np.float32)
    bq = np.asarray(inputs['bq'], np.float32)
    qe = np.asarray(inputs['query_embedding'], np.float32).reshape(DIM)
    beff = (bq + qe) * SCALE
    f[:, C_BQ + 0] = beff[:128]
    f[:, C_BQ + 1] = beff[128:]
    bkv = np.asarray(inputs['bkv'], np.float32)
    for i in range(4):
        f[:, C_BKV + i] = bkv[i * 128:(i + 1) * 128]
    bsr = np.asarray(inputs['bsr'], np.float32)
    f[:, C_BSR + 0] = bsr[:128]
    f[:, C_BSR + 1] = bsr[128:]
    bproj = np.asarray(inputs['bproj'], np.float32)
    f[:, C_BPROJ + 0] = bproj[:128]
    f[:, C_BPROJ + 1] = bproj[128:]
    g = np.asarray(inputs['ln_g'], np.float32)
    bb = np.asarray(inputs['ln_b'], np.float32)
    f[:, C_LNG + 0] = g[:128]
    f[:, C_LNG + 1] = g[128:]
    f[:, C_LNB + 0] = bb[:128]
    f[:, C_LNB + 1] = bb[128:]
    pbl = np.asarray(inputs['pos_bias_local'], np.float32)   # [NH, L]
    lb = np.asarray(inputs['learnable_bias'], np.float32).reshape(NH, L)
    for half in range(2):
        for l in range(L):
            for h4 in range(4):
                f[half * 36 + l * 4 + h4, C_PBL] = pbl[half * 4 + h4, l]
    pbp = np.asarray(inputs['pos_bias_pool'], np.float32)    # [NH, PL]
    for hh in range(4):
        for h2 in range(2):
            f[h2 * PL:(h2 + 1) * PL, C_PB2 + hh] = pbp[hh * 2 + h2]
    for h in range(NH):
        half, h4 = divmod(h, 4)
        for l in range(L):
            f[h, C_E72 + half * 36 + l * 4 + h4] = 1.0
    for hh in range(4):
        for h2 in range(2):
            f[hh * 2 + h2, C_E98 + hh * 98 + h2 * PL: C_E98 + hh * 98 + (h2 + 1) * PL] = 1.0
    f[:, C_IDF:C_IDF + 128] = np.eye(128, dtype=np.float32)

    b = np.zeros((128, NBF), np.float32)
    for half in range(2):
        for l in range(L):
            base = K_BO72 + (half * L + l) * 72
            for h4 in range(4):
                b[h4 * 32:(h4 + 1) * 32, base + half * 36 + l * 4 + h4] = 1.0
    for half in range(2):
        for l in range(L):
            for h4 in range(4):
                b[half * 36 + l * 4 + h4, K_BL + half * 4 + h4] = 1.0
    lt = np.asarray(inputs['learnable_tokens'], np.float32)  # [NH, HD, L]
    for half in range(2):
        for h4 in range(4):
            for l in range(L):
                b[h4 * 32:(h4 + 1) * 32, K_LT4 + half * 72 + half * 36 + l * 4 + h4] = \
                    lt[half * 4 + h4, :, l] / SCALE
    b[:, K_I128:K_I128 + 128] = np.eye(128, dtype=np.float32)
    for hh in range(4):
        b[0:PL, K_ON8 + hh * 8 + hh * 2] = 1.0
        b[PL:2 * PL, K_ON8 + hh * 8 + hh * 2 + 1] = 1.0
    # masks: value 1 where window position valid.  3 chunk classes.
    for ci, cls in enumerate(('top', 'mid', 'bot')):
        m = np.zeros((72, CH), np.float32)
        for li, (di, dj) in enumerate([(a, c) for a in (-1, 0, 1) for c in (-1, 0, 1)]):
            for n in range(CH):
                r, cc = divmod(n, W)
                rg = r if cls == 'top' else (48 + r if cls == 'bot' else 8 + r)
                ok = (0 <= rg + di < H) and (0 <= cc + dj < W)
                if cls == 'mid':
                    ok = (0 <= cc + dj < W)
                if ok:
                    for half in range(2):
                        h4s = half * 36 + li * 4
                        m[h4s:h4s + 4, n] = 1.0
        b[0:72, K_MASK + ci * CH:K_MASK + (ci + 1) * CH] = m

    er = np.zeros((8, 72 + 4 * 98), np.float32)
    er[:, 0:72] = f[0:8, C_E72:C_E72 + 72]
    for hh in range(4):
        er[:, 72 + hh * 98:72 + (hh + 1) * 98] = f[0:8, C_E98 + hh * 98:C_E98 + (hh + 1) * 98]
    consts = {
        'er_d': er,
        'cf32': f,
        'cbf16': b.astype(ml_dtypes.bfloat16),
        'Wq_d': np.asarray(inputs['Wq'], np.float32),
        'Wkv_d': np.asarray(inputs['Wkv'], np.float32),
        'Wsr_d': np.asarray(inputs['Wsr'], np.float32),
        'Wproj_d': np.asarray(inputs['Wproj'], np.float32),
    }
    return consts


def _emit(nc, tc, io):
    from contextlib import ExitStack
    ctx = ExitStack()
    io['_ctx'] = ctx
    x_d, out_d = io['x_sh'], io['out']
    cf_d, cb_d = io['cf32'], io['cbf16']
    wq_d, wkv_d, wsr_d, wproj_d = io['Wq_d'], io['Wkv_d'], io['Wsr_d'], io['Wproj_d']

    pers = ctx.enter_context(tc.tile_pool(name="pers", bufs=1))
    stream = ctx.enter_context(tc.tile_pool(name="stream", bufs=3))
    chunkp = ctx.enter_context(tc.tile_pool(name="chunkp", bufs=2))
    prodp = ctx.enter_context(tc.tile_pool(name="prodp", bufs=4))
    pp_big = ctx.enter_context(tc.tile_pool(name="pp_big", bufs=2, space="PSUM"))
    pp_sm = ctx.enter_context(tc.tile_pool(name="pp_sm", bufs=3, space="PSUM"))
    pp_xl = ctx.enter_context(tc.tile_pool(name="pp_xl", bufs=2, space="PSUM"))
    dramp = ctx.enter_context(tc.tile_pool(name="dramp", bufs=2, space="DRAM"))

    # ---- constants ----
    cf = pers.tile([128, NF32], F32, name="cf")
    nc.sync.dma_start(cf[:], cf_d[:])
    er = pers.tile([8, 72 + 4 * 98], F32R, name="er")
    nc.sync.dma_start(er[:], io['er_d'][:])
    cb = pers.tile([128, NBF], BF16, name="cb")
    nc.sync.dma_start(cb[:], cb_d[:])
    IDF = cf[:, C_IDF:C_IDF + 128]
    I128 = cb[:, K_I128:K_I128 + 128]
    BL = cb[0:72, K_BL:K_BL + 8]

    # ---- dense weights to SBUF (fp32) ----
    def wtiles(dram, dout, nm):
        ts = []
        for ko in range(2):
            row = []
            for mo in range(dout // 128):
                t = pers.tile([128, 128], F32R, name=f"{nm}{ko}{mo}")
                nc.sync.dma_start(t[:], dram[ko * 128:(ko + 1) * 128, mo * 128:(mo + 1) * 128])
                row.append(t)
            ts.append(row)
        return ts
    Wq = wtiles(wq_d, 256, "wq")
    Wkv = wtiles(wkv_d, 512, "wkv")
    Wsr = wtiles(wsr_d, 256, "wsr")
    Wproj = wtiles(wproj_d, 256, "wpj")

    # ---- persistent activations ----
    xT = [pers.tile([128, N], F32R, name=f"xT{i}") for i in range(2)]
    qT = [pers.tile([128, N], BF16, name=f"qT{i}") for i in range(2)]
    kT = [pers.tile([128, NKV], BF16, name=f"kT{i}") for i in range(2)]
    vT = [pers.tile([128, NKV], BF16, name=f"vT{i}") for i in range(2)]
    xsT = [pers.tile([128, N], BF16, name=f"xsT{i}") for i in range(2)]
    for t in kT + vT:
        nc.gpsimd.memset(t[:, 0:PAD], 0.0)
        nc.gpsimd.memset(t[:, PAD + N:NKV], 0.0)

    # ---- load x and transpose to xT (fp32) ----
    tss = [128] * 24 + [64]
    for ti in range(25):
        ts = tss[ti]
        xin = stream.tile([128, 256], F32, tag="xin")
        nc.sync.dma_start(xin[0:ts, :], x_d[ti * 128:ti * 128 + ts, :])
        for ko in range(2):
            ps = pp_big.tile([128, 128], F32, tag="big")
            nc.tensor.transpose(ps[0:128, 0:ts], xin[0:ts, ko * 128:(ko + 1) * 128], IDF[0:ts, 0:ts])
            nc.scalar.copy(xT[ko][:, ti * 128:ti * 128 + ts], ps[0:128, 0:ts])

    def dense(mo_tiles, wt, rhs_tiles, c):
        """returns list of psum tiles [128, CH] for each mo"""
        outs = []
        for mo in range(mo_tiles):
            ps = pp_big.tile([128, CH], F32, tag="big")
            for ko in range(2):
                nc.tensor.matmul(ps[:], wt[ko][mo][:],
                                 rhs_tiles[ko][:, c * CH:(c + 1) * CH],
                                 start=(ko == 0), stop=(ko == 1))
            outs.append(ps)
        return outs

    # ---- sr branch first (keeps Gelu table resident), then pooling ----
    for c in range(NCH):
        for mo, ps in enumerate(dense(2, Wsr, xT, c)):
            nc.scalar.activation(xsT[mo][:, c * CH:(c + 1) * CH], ps[:], AF.Gelu,
                                 bias=cf[:, C_BSR + mo:C_BSR + mo + 1])

    # ---- q / kv dense for whole image ----
    for c in range(NCH):
        for mo, ps in enumerate(dense(2, Wq, xT, c)):
            nc.scalar.activation(qT[mo][:, c * CH:(c + 1) * CH], ps[:], AF.Identity,
                                 bias=cf[:, C_BQ + mo:C_BQ + mo + 1], scale=SCALE)
        for mo, ps in enumerate(dense(4, Wkv, xT, c)):
            dst = kT[mo] if mo < 2 else vT[mo - 2]
            nc.scalar.activation(dst[:, PAD + c * CH:PAD + (c + 1) * CH], ps[:], AF.Identity,
                                 bias=cf[:, C_BKV + mo:C_BKV + mo + 1])

    # ---- pooling: s = sum over 8x8 blocks of xsT ----
    s_sum = (pers.tile([128, PL], F32, name="s_sum0"), pers.tile([128, PL], F32, name="s_sum1"))
    for half in range(2):
        for pi in range(PH):
            ap = xsT[half][:, pi * CH:(pi + 1) * CH].rearrange("p (r pj c) -> p pj r c", r=8, pj=PW, c=8)
            nc.vector.tensor_reduce(s_sum[half][:, pi * PW:(pi + 1) * PW], ap,
                                    axis=mybir.AxisListType.XY, op=mybir.AluOpType.add)

    # LN stats via PE ones-reduction
    ones1 = pers.tile([128, 1], F32, name="ones1")
    nc.gpsimd.memset(ones1[:], 1.0)
    onesr = pers.tile([1, 128], F32, name="onesr")
    nc.gpsimd.memset(onesr[:], 1.0)
    ps_mu = pp_sm.tile([1, PL], F32, tag="sm")
    for half in range(2):
        nc.tensor.matmul(ps_mu[:], ones1[:], s_sum[half][:], start=(half == 0), stop=(half == 1))
    sq = [stream.tile([128, PL], F32, tag="sq", name=f"sq{i}") for i in range(2)]
    for half in range(2):
        nc.scalar.square(sq[half][:], s_sum[half][:])
    ps_m2 = pp_sm.tile([1, PL], F32, tag="sm")
    for half in range(2):
        nc.tensor.matmul(ps_m2[:], ones1[:], sq[half][:], start=(half == 0), stop=(half == 1))
    mu = pers.tile([1, PL], F32, name="mu")
    nc.vector.tensor_scalar_mul(mu[:], ps_mu[:], 1.0 / DIM)
    var = pers.tile([1, PL], F32, name="var")
    nc.vector.tensor_scalar_mul(var[:], ps_m2[:], 1.0 / DIM)
    musq = pers.tile([1, PL], F32, name="musq")
    nc.vector.tensor_mul(musq[:], mu[:], mu[:])
    nc.vector.tensor_sub(var[:], var[:], musq[:])
    nc.vector.tensor_scalar_add(var[:], var[:], LN_EPS_S)
    lnv = pers.tile([1, PL], F32, name="lnv")
    nc.scalar.activation(lnv[:], var[:], AF.Ln)
    nc.vector.tensor_scalar_mul(lnv[:], lnv[:], -0.5)
    rstd = pers.tile([1, PL], F32, name="rstd")
    nc.scalar.activation(rstd[:], lnv[:], AF.Exp)
    # broadcast mu/rstd to 128 partitions via PE (K=1 matmul with ones col)
    ps_mub = pp_sm.tile([128, PL], F32, tag="sm")
    nc.tensor.matmul(ps_mub[:], onesr[:], mu[:], start=True, stop=True)
    ps_rsb = pp_sm.tile([128, PL], F32, tag="sm")
    nc.tensor.matmul(ps_rsb[:], onesr[:], rstd[:], start=True, stop=True)
    xpn = [stream.tile([128, PL], F32, tag="xpn", name=f"xpn{i}") for i in range(2)]
    for half in range(2):
        nc.vector.tensor_sub(xpn[half][:], s_sum[half][:], ps_mub[:])
        nc.vector.tensor_mul(xpn[half][:], xpn[half][:], ps_rsb[:])
        nc.vector.tensor_scalar(xpn[half][:], xpn[half][:],
                                cf[:, C_LNG + half:C_LNG + half + 1],
                                cf[:, C_LNB + half:C_LNB + half + 1],
                                op0=mybir.AluOpType.mult, op1=mybir.AluOpType.add)
    # kvp = Wkv @ xpn + bkv -> k_pool/v_pool bf16 [128, 49] tiles (plain fp32 GEMM)
    wkvf = []
    for ko in range(2):
        row = []
        for mo in range(4):
            t = pers.tile([128, 128], F32, name=f"wkvf{ko}{mo}")
            nc.sync.dma_start(t[:], wkv_d[ko * 128:(ko + 1) * 128, mo * 128:(mo + 1) * 128].bitcast(F32))
            row.append(t)
        wkvf.append(row)
    kvp = []
    for mo in range(4):
        ps = pp_sm.tile([128, PL], F32, tag="sm")
        for ko in range(2):
            nc.tensor.matmul(ps[:], wkvf[ko][mo][:], xpn[ko][:], start=(ko == 0), stop=(ko == 1))
        t = pers.tile([128, PL], BF16, name=f"kvp{mo}")
        nc.scalar.activation(t[:], ps[:], AF.Identity, bias=cf[:, C_BKV + mo:C_BKV + mo + 1])
        kvp.append(t)
    # kp2[hh]: [128, 98] lhsT (rows (hh%2)*64..+64 hold block-diag 2-head k_pool,
    # other rows zero so K can span the full qT half tile).  vp2[hh]: [98, 64].
    kp2, vp2 = [], []
    for hh in range(4):
        a = pers.tile([128, 98], BF16, name=f"kp2_{hh}")
        nc.gpsimd.memset(a[:], 0.0)
        kp2.append(a)
        b_ = pers.tile([98, 64], BF16, name=f"vp2_{hh}")
        nc.gpsimd.memset(b_[:], 0.0)
        vp2.append(b_)
    for tl in range(2):
        pst = pp_sm.tile([PL, 128], BF16, tag="smT", name="pst", bufs=1)
        nc.tensor.transpose(pst[:], kvp[2 + tl][:], I128)
        stg = stream.tile([PL, 128], BF16, tag="stg", name="stg")
        nc.scalar.copy(stg[:], pst[:])
        for ro in range(4):
            h = tl * 4 + ro
            hh, h2 = divmod(h, 2)
            nc.sync.dma_start(kp2[hh][(h % 4) * 32:(h % 4 + 1) * 32, h2 * PL:(h2 + 1) * PL],
                              kvp[tl][ro * 32:(ro + 1) * 32, :])
            nc.sync.dma_start(vp2[hh][h2 * PL:(h2 + 1) * PL, h2 * 32:(h2 + 1) * 32],
                              stg[0:PL, ro * 32:(ro + 1) * 32])

    # ---------------- attention main loop ----------------
    for c in range(NCH):
        c0 = c * CH
        mcls = 0 if c == 0 else (2 if c == NCH - 1 else 1)
        mask = cb[0:72, K_MASK + mcls * CH:K_MASK + (mcls + 1) * CH]

        # local qk products + d-reduction
        ps_lg = pp_sm.tile([72, CH], F32, tag="sm")
        nmm = 0
        for half in range(2):
            for li, off in enumerate(OFFS):
                pr = prodp.tile([128, CH], BF16, tag="pr")
                nc.vector.tensor_mul(pr[:], qT[half][:, c0:c0 + CH],
                                     kT[half][:, PAD + c0 + off:PAD + c0 + off + CH])
                base = K_BO72 + (half * L + li) * 72
                nc.tensor.matmul(ps_lg[:], cb[:, base:base + 72], pr[:],
                                 start=(nmm == 0), stop=(nmm == 17), skip_group_check=True)
                nmm += 1
        el = chunkp.tile([72, CH], BF16, tag="el")
        nc.scalar.activation(el[:], ps_lg[:], AF.Exp, bias=cf[0:72, C_PBL:C_PBL + 1])
        elm = chunkp.tile([72, CH], BF16, tag="elm")
        nc.vector.tensor_mul(elm[:], el[:], mask)
        ps_dl = pp_sm.tile([8, CH], F32, tag="sm")
        nc.tensor.matmul(ps_dl[:], BL, elm[:], start=True, stop=False,
                         skip_group_check=True)

        # pool scores + exp + pool denominator (accumulated onto ps_dl)
        eps = []
        for hh in range(4):
            ps_sp = pp_sm.tile([98, CH], F32, tag="sm")
            nc.tensor.matmul(ps_sp[:], kp2[hh][:], qT[hh // 2][:, c0:c0 + CH],
                             start=True, stop=True)
            ep = chunkp.tile([98, CH], BF16, tag=f"ep{hh}")
            nc.scalar.activation(ep[:], ps_sp[:], AF.Exp, bias=cf[0:98, C_PB2 + hh:C_PB2 + hh + 1])
            eps.append(ep)
            nc.tensor.matmul(ps_dl[:], cb[0:98, K_ON8 + hh * 8:K_ON8 + hh * 8 + 8], ep[:],
                             start=False, stop=(hh == 3), skip_group_check=True)

        # total denominator -> reciprocal
        rc = chunkp.tile([8, CH], F32, tag="rc")
        nc.vector.reciprocal_approx_fast(rc[:], ps_dl[:])

        # expand R to [72] (for W) and [98] (for pool AV)
        rcr = chunkp.tile([8, CH], F32R, tag="rcr")
        nc.scalar.copy(rcr[:], rc[:])
        ps_r72 = pp_sm.tile([72, CH], F32, tag="sm")
        nc.tensor.matmul(ps_r72[:], er[:, 0:72], rcr[:], start=True, stop=True)

        # W = qlt + elm * R72 (masked)
        wt = chunkp.tile([72, CH], F32, tag="wt")
        nc.vector.tensor_mul(wt[:], elm[:], ps_r72[:])
        ps_q = pp_sm.tile([72, CH], F32, tag="sm")
        for half in range(2):
            nc.tensor.matmul(ps_q[:], cb[:, K_LT4 + half * 72:K_LT4 + half * 72 + 72],
                             qT[half][:, c0:c0 + CH], start=(half == 0), stop=(half == 1),
                             skip_group_check=True)
        wb = chunkp.tile([72, CH], BF16, tag="wb")
        nc.vector.tensor_add(wb[:], wt[:], ps_q[:])
        wm = chunkp.tile([72, CH], BF16, tag="wm")
        nc.vector.tensor_mul(wm[:], wb[:], mask)

        # W round-trip through DRAM for head-broadcast
        wd = dramp.tile([72, 1, CH], BF16, tag="wd")
        nc.sync.dma_start(wd[:].rearrange("p o n -> p (o n)"), wm[:])

        # x_local + x_pool accumulate in psum
        ps_xl = [pp_xl.tile([128, CH], F32, tag="xl", name=f"ps_xl{i}") for i in range(2)]
        for half in range(2):
            for li, off in enumerate(OFFS):
                we = prodp.tile([128, CH], BF16, tag="we")
                src = wd[half * 36 + li * 4:half * 36 + li * 4 + 4, :, :].broadcast_to([4, 32, CH])
                nc.sync.dma_start(we[:], src)
                pl_ = prodp.tile([128, CH], BF16, tag="pl")
                nc.vector.tensor_mul(pl_[:], vT[half][:, PAD + c0 + off:PAD + c0 + off + CH], we[:])
                nc.tensor.matmul(ps_xl[half][:], I128, pl_[:], start=(li == 0), stop=False,
                                 skip_group_check=True)
        for hh in range(4):
            ps_r98 = pp_sm.tile([98, CH], F32, tag="sm")
            nc.tensor.matmul(ps_r98[:], er[:, 72 + hh * 98:72 + (hh + 1) * 98], rcr[:],
                             start=True, stop=True)
            epn = chunkp.tile([98, CH], BF16, tag=f"epn{hh}")
            nc.vector.tensor_mul(epn[:], eps[hh][:], ps_r98[:])
            nc.tensor.matmul(ps_xl[hh // 2][(hh % 2) * 64:(hh % 2) * 64 + 64, :],
                             vp2[hh][:], epn[:], start=False, stop=True,
                             skip_group_check=True)

        # project + transpose out
        xo = [chunkp.tile([128, CH], F32R, tag=f"xo{i}", name=f"xo{i}") for i in range(2)]
        for half in range(2):
            nc.scalar.copy(xo[half][:], ps_xl[half][:])
        for mo in range(2):
            ps_o = pp_big.tile([128, CH], F32, tag="big")
            for ko in range(2):
                nc.tensor.matmul(ps_o[:], Wproj[ko][mo][:],
                                 xo[ko][:], start=(ko == 0), stop=(ko == 1))
            pj = chunkp.tile([128, CH], F32, tag=f"pj{mo}")
            nc.scalar.activation(pj[:], ps_o[:], AF.Identity,
                                 bias=cf[:, C_BPROJ + mo:C_BPROJ + mo + 1])
            for bk in range(4):
                ps_t = pp_big.tile([112, 128], F32, tag="big")
                nc.tensor.transpose(ps_t[:], pj[:, bk * 112:(bk + 1) * 112], IDF)
                ot = stream.tile([112, 128], F32, tag="ot", name="ot")
                nc.scalar.copy(ot[:], ps_t[:])
                nc.sync.dma_start(out_d[c0 + bk * 112:c0 + (bk + 1) * 112, mo * 128:(mo + 1) * 128],
                                  ot[:])

    ctx.close()


_BUILD_CACHE = {}


def _build():
    if 'nc' in _BUILD_CACHE:
        return _BUILD_CACHE['nc']
    nc = bacc.Bacc("TRN2", target_bir_lowering=False, debug=False, num_devices=8)
    io = {
        'x_sh': nc.dram_tensor("x_sh", [N, DIM], F32, kind="ExternalInput").ap(),
        'cf32': nc.dram_tensor("cf32", [128, NF32], F32, kind="ExternalInput").ap(),
        'cbf16': nc.dram_tensor("cbf16", [128, NBF], BF16, kind="ExternalInput").ap(),
        'Wq_d': nc.dram_tensor("Wq_d", [DIM, DIM], F32R, kind="ExternalInput").ap(),
        'Wkv_d': nc.dram_tensor("Wkv_d", [DIM, 2 * DIM], F32R, kind="ExternalInput").ap(),
        'Wsr_d': nc.dram_tensor("Wsr_d", [DIM, DIM], F32R, kind="ExternalInput").ap(),
        'Wproj_d': nc.dram_tensor("Wproj_d", [DIM, DIM], F32R, kind="ExternalInput").ap(),
        'er_d': nc.dram_tensor("er_d", [8, 72 + 4 * 98], F32R, kind="ExternalInput").ap(),
        'out': nc.dram_tensor("out", [N, DIM], F32, kind="ExternalOutput").ap(),
    }
    with tile.TileContext(nc) as tc:
        _emit(nc, tc, io)
    nc.compile()
    _BUILD_CACHE['nc'] = nc
    return nc


def kernel(**inputs) -> np.ndarray:
    nc = _build()
    consts = _host_consts(inputs)
    x = np.asarray(inputs['x'], np.float32)
    in_maps = []
    for core in range(8):
        m = {'x_sh': np.ascontiguousarray(x[core])}
        m.update({k: np.ascontiguousarray(v) for k, v in consts.items()})
        in_maps.append(m)
    res = run_bass_kernel_spmd(nc, in_maps, core_ids=list(range(8)))
    out = np.stack([res.results[i]['out'] for i in range(8)], axis=0)
    return out.astype(np.float32)

